# revision 17
# baseline (speedup 1.0000x reference)
"""Trainium2 Bass kernel for DigitConvolutionalModel.

Model: x[B,784] -> reshape 28x28 -> 3x3 valid conv (weights conv_w) ->
[B,676] -> Linear(676,100)+relu -> Linear(100,10)+relu -> Linear(10,10).

The conv is linear, so it folds into the first Linear: W1f = C @ w1 where
C[784,676] is the conv unfold matrix. The whole model becomes a 3-layer MLP
784 -> 100 -> 10 -> 10 with relu between layers.

Sharding: pure data parallel, batch split across 8 cores (8192 rows each).

Precision: matmuls in bf16 (PE streams fp32 at 1/4 rate, bf16 at full
rate), accumulation in fp32 PSUM, biases + output in fp32. x is cast to
bf16 host-side — bit-identical to casting on device, but halves the HBM
traffic, which is what the ridge regime wants (DMA ~36us/core vs PE
~31us/core).

On-chip layout: activations stay feature-major ([features, batch] on SBUF
partitions) end to end, so every matmul uses the weights in natural [in,out]
layout as the stationary operand and the batch streams as the moving free
dim:
    h1T[100,n] = sum_k W1f[k,:].T @ xT[k,n]      (K chunks of <=128)
    h2T[10,n]  = w2.T @ relu(h1T+b1)
    yT[10,n]   = w3.T @ relu(h2T+b2) + b3
The [10, B] output is stored feature-major (2KB-contiguous DMA runs; a
batch-major store would emit 40-byte descriptors) and transposed on host.

x is staged host-side into the feature-major tiled layout the DMA wants:
per 512-batch supertile [128, 6, 512] (features 0..767, 6KB contiguous per
partition); the [16, 512] feature tails (768..783) of all supertiles are
preloaded in one DMA.
"""

import numpy as np
import ml_dtypes

import concourse.bacc as bacc
import concourse.tile as tile
from concourse.tile import add_dep_helper
from concourse import mybir
from concourse.bass_utils import run_bass_kernel_spmd

N_CORES = 8
B = 65536
BC = B // N_CORES  # 8192 rows per core
TN = 512           # batch columns per supertile
NT = BC // TN      # 16 supertiles per core
NKC = 6            # full 128-feature chunks (0..767)
KT = 16            # tail features (768..783)
NF = 784
H1 = 100
HO = 10
F32 = mybir.dt.float32
BF16 = mybir.dt.bfloat16
NP_BF16 = ml_dtypes.bfloat16


def _build_nc():
    nc = bacc.Bacc(None, target_bir_lowering=False)

    xt_main = nc.dram_tensor(
        "xt_main", [NT // 2, 128, 2, NKC, TN], BF16, kind="ExternalInput"
    )
    xt_tail = nc.dram_tensor("xt_tail", [KT, NT, TN], BF16, kind="ExternalInput")
    w1m = nc.dram_tensor("w1m", [128, NKC, H1], BF16, kind="ExternalInput")
    w1t = nc.dram_tensor("w1t", [KT, H1], BF16, kind="ExternalInput")
    b1 = nc.dram_tensor("b1", [H1, 1], F32, kind="ExternalInput")
    w2 = nc.dram_tensor("w2", [H1, HO], BF16, kind="ExternalInput")
    b2 = nc.dram_tensor("b2", [HO, 1], F32, kind="ExternalInput")
    w3 = nc.dram_tensor("w3", [HO, HO], BF16, kind="ExternalInput")
    b3w = nc.dram_tensor("b3w", [HO, TN], F32, kind="ExternalInput")
    yt = nc.dram_tensor("yt", [HO, BC], F32, kind="ExternalOutput")

    relu = mybir.ActivationFunctionType.Relu

    with tile.TileContext(nc) as tc:
        with (
            tc.tile_pool(name="const", bufs=1) as cpool,
            tc.tile_pool(name="io", bufs=4) as iopool,
            tc.tile_pool(name="act", bufs=4) as apool,
            tc.tile_pool(name="ps1", bufs=4, space="PSUM") as ps1,
            tc.tile_pool(name="ps2", bufs=2, space="PSUM") as ps2,
            tc.tile_pool(name="ps3", bufs=2, space="PSUM") as ps3,
        ):
# Weights/consts go on the scalar HWDGE queue-set so they don't
            # delay the batch-data stream on the sync queue-set.
            w1m_s = cpool.tile([128, NKC, H1], BF16, tag="w1m")
            nc.scalar.dma_start(w1m_s[:], w1m[:])
            w1t_s = cpool.tile([KT, H1], BF16, tag="w1t")
            nc.scalar.dma_start(w1t_s[:], w1t[:])
            xtl_s = cpool.tile([KT, NT, TN], BF16, tag="xtl")
            nc.scalar.dma_start(xtl_s[:], xt_tail[:])
            b1_s = cpool.tile([H1, 1], F32, tag="b1")
            nc.scalar.dma_start(b1_s[:], b1[:])
            w2_s = cpool.tile([H1, HO], BF16, tag="w2")
            nc.scalar.dma_start(w2_s[:], w2[:])
            b2_s = cpool.tile([HO, 1], F32, tag="b2")
            nc.scalar.dma_start(b2_s[:], b2[:])
            w3_s = cpool.tile([HO, HO], BF16, tag="w3")
            nc.scalar.dma_start(w3_s[:], w3[:])
            b3w_s = cpool.tile([HO, TN], F32, tag="b3w")
            nc.scalar.dma_start(b3w_s[:], b3w[:])

            # All real matmuls are chained with same-engine ordering deps
            # so the PE executes them exactly in emission order — required
            # for ldweights=False (weight reuse from the previous matmul).
            prev_mm = [None]

            def mm(out_ap, lhsT_ap, rhs_ap, start, stop, ldw=True):
                m = nc.tensor.matmul(out_ap, lhsT_ap, rhs_ap,
                                     start=start, stop=stop)
                if not ldw:
                    m.ins.ldweights = False
                if prev_mm[0] is not None:
                    add_dep_helper(m.ins, prev_mm[0], sync=False,
                                   reason="pe program order")
                prev_mm[0] = m.ins
                return m

            # Warmup: dense dummy matmuls fill the NEFF startup ramp
            # (~12us of instruction loads + first DMAs) so the PE's HAM
            # throttle reaches full clock before the first real matmul.
            wsc = cpool.tile([128, TN], BF16, tag="wsc")
            nc.gpsimd.memset(wsc[:], 0.0)
            wp0 = ps1.tile([H1, TN], F32, tag="p1")
            wp1 = ps1.tile([H1, TN], F32, tag="p1")
            wp = [wp0, wp1]
            wfirst = nc.tensor.matmul(wp[0][:], wsc[:, 0:H1], wsc[:],
                                      start=True, stop=True)
            for i in range(1, 24):
                w_mm = nc.tensor.matmul(wp[i % 2][:], wsc[:, 0:H1], wsc[:],
                                        start=True, stop=True)
                w_mm.ins.ldweights = False
                add_dep_helper(w_mm.ins, wfirst.ins, sync=False,
                               reason="warmup weight reuse")

            # Pipeline over supertile PAIRS: at pair p emit L1(p),
            # L2(p-1), L3(p-2). Within a pair the two supertiles share
            # every LDWEIGHTS via ldweights=False (chunk-outer order).
            NP = NT // 2
            h1s: dict[int, object] = {}
            h2s: dict[int, object] = {}
            for p in range(NP + 2):
                if p < NP:
                    t0 = 2 * p
                    xmp = iopool.tile([128, 2, NKC, TN], BF16, tag="xm")
                    nc.sync.dma_start(xmp[:], xt_main[p])

                    p1a = ps1.tile([H1, TN], F32, tag="p1")
                    p1b = ps1.tile([H1, TN], F32, tag="p1")
                    for k in range(NKC):
                        mm(p1a[:], w1m_s[:, k, :], xmp[:, 0, k, :],
                           start=(k == 0), stop=False)
                        mm(p1b[:], w1m_s[:, k, :], xmp[:, 1, k, :],
                           start=(k == 0), stop=False, ldw=False)
                    mm(p1a[:], w1t_s[:], xtl_s[:, t0, :],
                       start=False, stop=True)
                    mm(p1b[:], w1t_s[:], xtl_s[:, t0 + 1, :],
                       start=False, stop=True, ldw=False)
                    for j, p1 in ((0, p1a), (1, p1b)):
                        h1 = apool.tile([H1, TN], BF16, tag="h1")
                        nc.scalar.activation(h1[:], p1[:], relu,
                                             bias=b1_s[:, 0:1])
                        h1s[t0 + j] = h1

                if 1 <= p < NP + 1:
                    s0 = 2 * (p - 1)
                    p2a = ps2.tile([HO, TN], F32, tag="p2")
                    p2b = ps2.tile([HO, TN], F32, tag="p2")
                    mm(p2a[:], w2_s[:], h1s.pop(s0)[:], start=True, stop=True)
                    mm(p2b[:], w2_s[:], h1s.pop(s0 + 1)[:],
                       start=True, stop=True, ldw=False)
                    for j, p2 in ((0, p2a), (1, p2b)):
                        h2 = apool.tile([HO, TN], BF16, tag="h2")
                        nc.scalar.activation(h2[:], p2[:], relu,
                                             bias=b2_s[:, 0:1])
                        h2s[s0 + j] = h2

                if p >= 2:
                    s0 = 2 * (p - 2)
                    p3a = ps3.tile([HO, TN], F32, tag="p3")
                    p3b = ps3.tile([HO, TN], F32, tag="p3")
                    mm(p3a[:], w3_s[:], h2s.pop(s0)[:], start=True, stop=True)
                    mm(p3b[:], w3_s[:], h2s.pop(s0 + 1)[:],
                       start=True, stop=True, ldw=False)
                    ot = apool.tile([HO, 2, TN], F32, tag="ot")
                    nc.vector.tensor_add(ot[:, 0, :], p3a[:], b3w_s[:])
                    nc.vector.tensor_add(ot[:, 1, :], p3b[:], b3w_s[:])
                    nc.sync.dma_start(
                        yt[:, s0 * TN:(s0 + 2) * TN], ot[:]
                    )

    nc.compile()
    return nc


def _fold_conv_into_w1(conv_w: np.ndarray, w1: np.ndarray) -> np.ndarray:
    """W1f[784,100] such that x @ W1f == conv(x).reshape(B,676) @ w1."""
    c = np.zeros((NF, 26 * 26), dtype=np.float64)
    for di in range(3):
        for dj in range(3):
            ii, jj = np.meshgrid(np.arange(26), np.arange(26), indexing="ij")
            src = (ii + di) * 28 + (jj + dj)
            dst = ii * 26 + jj
            c[src.ravel(), dst.ravel()] += np.float64(conv_w[di, dj])
    return (c @ w1.astype(np.float64)).astype(np.float32)


def _prep_in_maps(x, conv_w, w1, b1, w2, b2, w3, b3):
    x = np.asarray(x, dtype=np.float32)
    conv_w = np.asarray(conv_w, dtype=np.float32)
    w1 = np.asarray(w1, dtype=np.float32)
    b1 = np.asarray(b1, dtype=np.float32)
    w2 = np.asarray(w2, dtype=np.float32)
    b2 = np.asarray(b2, dtype=np.float32)
    w3 = np.asarray(w3, dtype=np.float32)
    b3 = np.asarray(b3, dtype=np.float32)

    w1f = _fold_conv_into_w1(conv_w, w1)  # [784, 100]
    # main chunks: feature f = k*128 + p -> [128, NKC, H1]
    w1m = np.ascontiguousarray(
        w1f[: 128 * NKC].reshape(NKC, 128, H1).transpose(1, 0, 2)
    ).astype(NP_BF16)
    w1t = np.ascontiguousarray(w1f[128 * NKC:]).astype(NP_BF16)  # [16, 100]
    b1c = np.ascontiguousarray(b1.reshape(H1, 1))
    b2c = np.ascontiguousarray(b2.reshape(HO, 1))
    b3w = np.ascontiguousarray(np.broadcast_to(b3.reshape(HO, 1), (HO, TN)))

    shared = {
        "w1m": w1m, "w1t": w1t, "b1": b1c,
        "w2": np.ascontiguousarray(w2).astype(NP_BF16), "b2": b2c,
        "w3": np.ascontiguousarray(w3).astype(NP_BF16), "b3w": b3w,
    }

    xb = x.astype(NP_BF16)  # cast once, full batch
    NP2 = NT // 2
    in_maps = []
    for core in range(N_CORES):
        xc = xb[core * BC:(core + 1) * BC]  # [8192, 784] bf16
        # [NP2, feat, pair-half, TN] feature-major per supertile pair
        xct = xc.reshape(NP2, 2, TN, NF).transpose(0, 3, 1, 2)  # [NP2, NF, 2, TN]
        xt_main = np.ascontiguousarray(
            xct[:, : 128 * NKC].reshape(NP2, NKC, 128, 2, TN)
            .transpose(0, 2, 3, 1, 4)
        )  # [NP2, 128, 2, NKC, TN]
        # tails: [KT, NT, TN] with t = 2*pair + half
        xt_tail = np.ascontiguousarray(
            xct[:, 128 * NKC:].transpose(1, 0, 2, 3).reshape(KT, NT, TN)
        )
        in_maps.append({"xt_main": xt_main, "xt_tail": xt_tail, **shared})
    return in_maps


_NC = None


def _get_nc():
    global _NC
    if _NC is None:
        _NC = _build_nc()
    return _NC


def kernel(x, conv_w, w1, b1, w2, b2, w3, b3):
    in_maps = _prep_in_maps(x, conv_w, w1, b1, w2, b2, w3, b3)
    nc = _get_nc()
    res = run_bass_kernel_spmd(nc, in_maps, core_ids=list(range(N_CORES)))
    out = np.empty((B, HO), dtype=np.float32)
    for i in range(N_CORES):
        out[i * BC:(i + 1) * BC] = res.results[i]["yt"].T
    return out


if __name__ == "__main__":
    rng = np.random.default_rng(0)
    inputs = {
        "x": rng.standard_normal((B, NF), dtype=np.float32),
        "conv_w": np.ones((3, 3), dtype=np.float32),
        "w1": (rng.standard_normal((676, H1)) * 0.04).astype(np.float32),
        "b1": np.zeros(H1, dtype=np.float32),
        "w2": (rng.standard_normal((H1, HO)) * 0.1).astype(np.float32),
        "b2": np.zeros(HO, dtype=np.float32),
        "w3": (rng.standard_normal((HO, HO)) * 0.3).astype(np.float32),
        "b3": np.zeros(HO, dtype=np.float32),
    }
    out = kernel(**inputs)
    print(out.shape, out.dtype)


# revision 18
# speedup vs baseline: 1.0229x; 1.0229x over previous
"""Trainium2 Bass kernel for DigitConvolutionalModel.

Model: x[B,784] -> reshape 28x28 -> 3x3 valid conv (weights conv_w) ->
[B,676] -> Linear(676,100)+relu -> Linear(100,10)+relu -> Linear(10,10).

The conv is linear, so it folds into the first Linear: W1f = C @ w1 where
C[784,676] is the conv unfold matrix. The whole model becomes a 3-layer MLP
784 -> 100 -> 10 -> 10 with relu between layers.

Sharding: pure data parallel, batch split across 8 cores (8192 rows each).

Precision: matmuls in bf16 (PE streams fp32 at 1/4 rate, bf16 at full
rate), accumulation in fp32 PSUM, biases + output in fp32. x is cast to
bf16 host-side — bit-identical to casting on device, but halves the HBM
traffic, which is what the ridge regime wants (DMA ~36us/core vs PE
~31us/core).

On-chip layout: activations stay feature-major ([features, batch] on SBUF
partitions) end to end, so every matmul uses the weights in natural [in,out]
layout as the stationary operand and the batch streams as the moving free
dim:
    h1T[100,n] = sum_k W1f[k,:].T @ xT[k,n]      (K chunks of <=128)
    h2T[10,n]  = w2.T @ relu(h1T+b1)
    yT[10,n]   = w3.T @ relu(h2T+b2) + b3
The [10, B] output is stored feature-major (2KB-contiguous DMA runs; a
batch-major store would emit 40-byte descriptors) and transposed on host.

x is staged host-side into the feature-major tiled layout the DMA wants:
per 512-batch supertile [128, 6, 512] (features 0..767, 6KB contiguous per
partition); the [16, 512] feature tails (768..783) of all supertiles are
preloaded in one DMA.
"""

import numpy as np
import ml_dtypes

import concourse.bacc as bacc
import concourse.tile as tile
from concourse.tile import add_dep_helper
from concourse import mybir
from concourse.bass_utils import run_bass_kernel_spmd

N_CORES = 8
B = 65536
BC = B // N_CORES  # 8192 rows per core
TN = 512           # batch columns per supertile
NT = BC // TN      # 16 supertiles per core
NKC = 6            # full 128-feature chunks (0..767)
KT = 16            # tail features (768..783)
NF = 784
H1 = 100
HO = 10
F32 = mybir.dt.float32
BF16 = mybir.dt.bfloat16
NP_BF16 = ml_dtypes.bfloat16


def _build_nc():
    nc = bacc.Bacc(None, target_bir_lowering=False)

    xt_main = nc.dram_tensor(
        "xt_main", [NT // 2, 128, 2, NKC, TN], BF16, kind="ExternalInput"
    )
    xt_tail = nc.dram_tensor("xt_tail", [KT, NT, TN], BF16, kind="ExternalInput")
    w1m = nc.dram_tensor("w1m", [128, NKC, H1], BF16, kind="ExternalInput")
    w1t = nc.dram_tensor("w1t", [KT, H1], BF16, kind="ExternalInput")
    b1 = nc.dram_tensor("b1", [H1, 1], F32, kind="ExternalInput")
    w2 = nc.dram_tensor("w2", [H1, HO], BF16, kind="ExternalInput")
    b2 = nc.dram_tensor("b2", [HO, 1], F32, kind="ExternalInput")
    w3 = nc.dram_tensor("w3", [HO, HO], BF16, kind="ExternalInput")
    b3w = nc.dram_tensor("b3w", [HO, TN], F32, kind="ExternalInput")
    yt = nc.dram_tensor("yt", [HO, BC], F32, kind="ExternalOutput")

    relu = mybir.ActivationFunctionType.Relu

    with tile.TileContext(nc) as tc:
        with (
            tc.tile_pool(name="const", bufs=1) as cpool,
            tc.tile_pool(name="io", bufs=6) as iopool,
            tc.tile_pool(name="act", bufs=4) as apool,
            tc.tile_pool(name="ps1", bufs=4, space="PSUM") as ps1,
            tc.tile_pool(name="ps2", bufs=2, space="PSUM") as ps2,
            tc.tile_pool(name="ps3", bufs=2, space="PSUM") as ps3,
        ):
# Weights/consts go on the scalar HWDGE queue-set so they don't
            # delay the batch-data stream on the sync queue-set.
            w1m_s = cpool.tile([128, NKC, H1], BF16, tag="w1m")
            nc.scalar.dma_start(w1m_s[:], w1m[:])
            w1t_s = cpool.tile([KT, H1], BF16, tag="w1t")
            nc.scalar.dma_start(w1t_s[:], w1t[:])
            xtl_s = cpool.tile([KT, NT, TN], BF16, tag="xtl")
            nc.scalar.dma_start(xtl_s[:], xt_tail[:])
            b1_s = cpool.tile([H1, 1], F32, tag="b1")
            nc.scalar.dma_start(b1_s[:], b1[:])
            w2_s = cpool.tile([H1, HO], BF16, tag="w2")
            nc.scalar.dma_start(w2_s[:], w2[:])
            b2_s = cpool.tile([HO, 1], F32, tag="b2")
            nc.scalar.dma_start(b2_s[:], b2[:])
            w3_s = cpool.tile([HO, HO], BF16, tag="w3")
            nc.scalar.dma_start(w3_s[:], w3[:])
            b3w_s = cpool.tile([HO, TN], F32, tag="b3w")
            nc.scalar.dma_start(b3w_s[:], b3w[:])

            # All real matmuls are chained with same-engine ordering deps
            # so the PE executes them exactly in emission order — required
            # for ldweights=False (weight reuse from the previous matmul).
            prev_mm = [None]

            def mm(out_ap, lhsT_ap, rhs_ap, start, stop, ldw=True):
                m = nc.tensor.matmul(out_ap, lhsT_ap, rhs_ap,
                                     start=start, stop=stop)
                if not ldw:
                    m.ins.ldweights = False
                if prev_mm[0] is not None:
                    add_dep_helper(m.ins, prev_mm[0], sync=False,
                                   reason="pe program order")
                prev_mm[0] = m.ins
                return m

            # Warmup: dense dummy matmuls fill the NEFF startup ramp
            # (~12us of instruction loads + first DMAs) so the PE's HAM
            # throttle reaches full clock before the first real matmul.
            wsc = cpool.tile([128, TN], BF16, tag="wsc")
            nc.gpsimd.memset(wsc[:], 0.0)
            wp0 = ps1.tile([H1, TN], F32, tag="p1")
            wp1 = ps1.tile([H1, TN], F32, tag="p1")
            wp = [wp0, wp1]
            wfirst = nc.tensor.matmul(wp[0][:], wsc[:, 0:H1], wsc[:],
                                      start=True, stop=True)
            for i in range(1, 24):
                w_mm = nc.tensor.matmul(wp[i % 2][:], wsc[:, 0:H1], wsc[:],
                                        start=True, stop=True)
                w_mm.ins.ldweights = False
                add_dep_helper(w_mm.ins, wfirst.ins, sync=False,
                               reason="warmup weight reuse")

            # Pipeline over supertile PAIRS: at pair p emit L1(p),
            # L2(p-1), L3(p-2). Within a pair the two supertiles share
            # every LDWEIGHTS via ldweights=False (chunk-outer order).
            NP = NT // 2
            h1s: dict[int, object] = {}
            h2s: dict[int, object] = {}
            for p in range(NP + 2):
                if p < NP:
                    t0 = 2 * p
                    xmp = iopool.tile([128, 2, NKC, TN], BF16, tag="xm")
                    nc.sync.dma_start(xmp[:], xt_main[p])

                    p1a = ps1.tile([H1, TN], F32, tag="p1")
                    p1b = ps1.tile([H1, TN], F32, tag="p1")
                    for k in range(NKC):
                        mm(p1a[:], w1m_s[:, k, :], xmp[:, 0, k, :],
                           start=(k == 0), stop=False)
                        mm(p1b[:], w1m_s[:, k, :], xmp[:, 1, k, :],
                           start=(k == 0), stop=False, ldw=False)
                    mm(p1a[:], w1t_s[:], xtl_s[:, t0, :],
                       start=False, stop=True)
                    mm(p1b[:], w1t_s[:], xtl_s[:, t0 + 1, :],
                       start=False, stop=True, ldw=False)
                    for j, p1 in ((0, p1a), (1, p1b)):
                        h1 = apool.tile([H1, TN], BF16, tag="h1")
                        nc.scalar.activation(h1[:], p1[:], relu,
                                             bias=b1_s[:, 0:1])
                        h1s[t0 + j] = h1

                if 1 <= p < NP + 1:
                    s0 = 2 * (p - 1)
                    p2a = ps2.tile([HO, TN], F32, tag="p2")
                    p2b = ps2.tile([HO, TN], F32, tag="p2")
                    mm(p2a[:], w2_s[:], h1s.pop(s0)[:], start=True, stop=True)
                    mm(p2b[:], w2_s[:], h1s.pop(s0 + 1)[:],
                       start=True, stop=True, ldw=False)
                    for j, p2 in ((0, p2a), (1, p2b)):
                        h2 = apool.tile([HO, TN], BF16, tag="h2")
                        nc.scalar.activation(h2[:], p2[:], relu,
                                             bias=b2_s[:, 0:1])
                        h2s[s0 + j] = h2

                if p >= 2:
                    s0 = 2 * (p - 2)
                    p3a = ps3.tile([HO, TN], F32, tag="p3")
                    p3b = ps3.tile([HO, TN], F32, tag="p3")
                    mm(p3a[:], w3_s[:], h2s.pop(s0)[:], start=True, stop=True)
                    mm(p3b[:], w3_s[:], h2s.pop(s0 + 1)[:],
                       start=True, stop=True, ldw=False)
                    ot = apool.tile([HO, 2, TN], F32, tag="ot")
                    nc.vector.tensor_add(ot[:, 0, :], p3a[:], b3w_s[:])
                    nc.vector.tensor_add(ot[:, 1, :], p3b[:], b3w_s[:])
                    nc.sync.dma_start(
                        yt[:, s0 * TN:(s0 + 2) * TN], ot[:]
                    )

    nc.compile()
    return nc


def _fold_conv_into_w1(conv_w: np.ndarray, w1: np.ndarray) -> np.ndarray:
    """W1f[784,100] such that x @ W1f == conv(x).reshape(B,676) @ w1."""
    c = np.zeros((NF, 26 * 26), dtype=np.float64)
    for di in range(3):
        for dj in range(3):
            ii, jj = np.meshgrid(np.arange(26), np.arange(26), indexing="ij")
            src = (ii + di) * 28 + (jj + dj)
            dst = ii * 26 + jj
            c[src.ravel(), dst.ravel()] += np.float64(conv_w[di, dj])
    return (c @ w1.astype(np.float64)).astype(np.float32)


def _prep_in_maps(x, conv_w, w1, b1, w2, b2, w3, b3):
    x = np.asarray(x, dtype=np.float32)
    conv_w = np.asarray(conv_w, dtype=np.float32)
    w1 = np.asarray(w1, dtype=np.float32)
    b1 = np.asarray(b1, dtype=np.float32)
    w2 = np.asarray(w2, dtype=np.float32)
    b2 = np.asarray(b2, dtype=np.float32)
    w3 = np.asarray(w3, dtype=np.float32)
    b3 = np.asarray(b3, dtype=np.float32)

    w1f = _fold_conv_into_w1(conv_w, w1)  # [784, 100]
    # main chunks: feature f = k*128 + p -> [128, NKC, H1]
    w1m = np.ascontiguousarray(
        w1f[: 128 * NKC].reshape(NKC, 128, H1).transpose(1, 0, 2)
    ).astype(NP_BF16)
    w1t = np.ascontiguousarray(w1f[128 * NKC:]).astype(NP_BF16)  # [16, 100]
    b1c = np.ascontiguousarray(b1.reshape(H1, 1))
    b2c = np.ascontiguousarray(b2.reshape(HO, 1))
    b3w = np.ascontiguousarray(np.broadcast_to(b3.reshape(HO, 1), (HO, TN)))

    shared = {
        "w1m": w1m, "w1t": w1t, "b1": b1c,
        "w2": np.ascontiguousarray(w2).astype(NP_BF16), "b2": b2c,
        "w3": np.ascontiguousarray(w3).astype(NP_BF16), "b3w": b3w,
    }

    xb = x.astype(NP_BF16)  # cast once, full batch
    NP2 = NT // 2
    in_maps = []
    for core in range(N_CORES):
        xc = xb[core * BC:(core + 1) * BC]  # [8192, 784] bf16
        # [NP2, feat, pair-half, TN] feature-major per supertile pair
        xct = xc.reshape(NP2, 2, TN, NF).transpose(0, 3, 1, 2)  # [NP2, NF, 2, TN]
        xt_main = np.ascontiguousarray(
            xct[:, : 128 * NKC].reshape(NP2, NKC, 128, 2, TN)
            .transpose(0, 2, 3, 1, 4)
        )  # [NP2, 128, 2, NKC, TN]
        # tails: [KT, NT, TN] with t = 2*pair + half
        xt_tail = np.ascontiguousarray(
            xct[:, 128 * NKC:].transpose(1, 0, 2, 3).reshape(KT, NT, TN)
        )
        in_maps.append({"xt_main": xt_main, "xt_tail": xt_tail, **shared})
    return in_maps


_NC = None


def _get_nc():
    global _NC
    if _NC is None:
        _NC = _build_nc()
    return _NC


def kernel(x, conv_w, w1, b1, w2, b2, w3, b3):
    in_maps = _prep_in_maps(x, conv_w, w1, b1, w2, b2, w3, b3)
    nc = _get_nc()
    res = run_bass_kernel_spmd(nc, in_maps, core_ids=list(range(N_CORES)))
    out = np.empty((B, HO), dtype=np.float32)
    for i in range(N_CORES):
        out[i * BC:(i + 1) * BC] = res.results[i]["yt"].T
    return out


if __name__ == "__main__":
    rng = np.random.default_rng(0)
    inputs = {
        "x": rng.standard_normal((B, NF), dtype=np.float32),
        "conv_w": np.ones((3, 3), dtype=np.float32),
        "w1": (rng.standard_normal((676, H1)) * 0.04).astype(np.float32),
        "b1": np.zeros(H1, dtype=np.float32),
        "w2": (rng.standard_normal((H1, HO)) * 0.1).astype(np.float32),
        "b2": np.zeros(HO, dtype=np.float32),
        "w3": (rng.standard_normal((HO, HO)) * 0.3).astype(np.float32),
        "b3": np.zeros(HO, dtype=np.float32),
    }
    out = kernel(**inputs)
    print(out.shape, out.dtype)


# revision 19
# speedup vs baseline: 1.1226x; 1.0975x over previous
"""Trainium2 Bass kernel for DigitConvolutionalModel.

Model: x[B,784] -> reshape 28x28 -> 3x3 valid conv (weights conv_w) ->
[B,676] -> Linear(676,100)+relu -> Linear(100,10)+relu -> Linear(10,10).

The conv is linear, so it folds into the first Linear: W1f = C @ w1 where
C[784,676] is the conv unfold matrix. The whole model becomes a 3-layer MLP
784 -> 100 -> 10 -> 10 with relu between layers.

Sharding: pure data parallel, batch split across 8 cores (8192 rows each).

Precision: matmuls in bf16 (PE streams fp32 at 1/4 rate, bf16 at full
rate), accumulation in fp32 PSUM, biases + output in fp32. x is cast to
bf16 host-side — bit-identical to casting on device, but halves the HBM
traffic, which is what the ridge regime wants (DMA ~36us/core vs PE
~31us/core).

On-chip layout: activations stay feature-major ([features, batch] on SBUF
partitions) end to end, so every matmul uses the weights in natural [in,out]
layout as the stationary operand and the batch streams as the moving free
dim:
    h1T[100,n] = sum_k W1f[k,:].T @ xT[k,n]      (K chunks of <=128)
    h2T[10,n]  = w2.T @ relu(h1T+b1)
    yT[10,n]   = w3.T @ relu(h2T+b2) + b3
The [10, B] output is stored feature-major (2KB-contiguous DMA runs; a
batch-major store would emit 40-byte descriptors) and transposed on host.

x is staged host-side into the feature-major tiled layout the DMA wants:
per 512-batch supertile [128, 6, 512] (features 0..767, 6KB contiguous per
partition); the [16, 512] feature tails (768..783) of all supertiles are
preloaded in one DMA.
"""

import numpy as np
import ml_dtypes

import concourse.bacc as bacc
import concourse.tile as tile
from concourse.tile import add_dep_helper
from concourse import mybir
from concourse.bass_utils import run_bass_kernel_spmd

N_CORES = 8
B = 65536
BC = B // N_CORES  # 8192 rows per core
TN = 512           # batch columns per supertile
NT = BC // TN      # 16 supertiles per core
NKC = 6            # full 128-feature chunks (0..767)
KT = 16            # tail features (768..783)
NF = 784
H1 = 100
HO = 10
F32 = mybir.dt.float32
BF16 = mybir.dt.bfloat16
NP_BF16 = ml_dtypes.bfloat16


def _build_nc():
    nc = bacc.Bacc(None, target_bir_lowering=False)

    xt_main = nc.dram_tensor(
        "xt_main", [NT // 2, 128, 2, NKC, TN], BF16, kind="ExternalInput"
    )
    xt_tail = nc.dram_tensor("xt_tail", [KT, NT, TN], BF16, kind="ExternalInput")
    w1m = nc.dram_tensor("w1m", [128, NKC, H1], BF16, kind="ExternalInput")
    w1t = nc.dram_tensor("w1t", [KT, H1], BF16, kind="ExternalInput")
    b1 = nc.dram_tensor("b1", [H1, 1], F32, kind="ExternalInput")
    w2 = nc.dram_tensor("w2", [H1, HO], BF16, kind="ExternalInput")
    b2 = nc.dram_tensor("b2", [HO, 1], F32, kind="ExternalInput")
    w3 = nc.dram_tensor("w3", [HO, HO], BF16, kind="ExternalInput")
    b3w = nc.dram_tensor("b3w", [HO, TN], F32, kind="ExternalInput")
    yt = nc.dram_tensor("yt", [HO, BC], F32, kind="ExternalOutput")

    relu = mybir.ActivationFunctionType.Relu

    with tile.TileContext(nc) as tc:
        with (
            tc.tile_pool(name="const", bufs=1) as cpool,
            tc.tile_pool(name="io", bufs=6) as iopool,
            tc.tile_pool(name="act", bufs=4) as apool,
            tc.tile_pool(name="ps1", bufs=4, space="PSUM") as ps1,
            tc.tile_pool(name="ps2", bufs=2, space="PSUM") as ps2,
            tc.tile_pool(name="ps3", bufs=2, space="PSUM") as ps3,
        ):
# Weights/consts go on the scalar HWDGE queue-set so they don't
            # delay the batch-data stream on the sync queue-set.
            w1m_s = cpool.tile([128, NKC, H1], BF16, tag="w1m")
            nc.scalar.dma_start(w1m_s[:], w1m[:])
            w1t_s = cpool.tile([KT, H1], BF16, tag="w1t")
            nc.scalar.dma_start(w1t_s[:], w1t[:])
            xtl_s = cpool.tile([KT, NT, TN], BF16, tag="xtl")
            nc.scalar.dma_start(xtl_s[:], xt_tail[:])
            b1_s = cpool.tile([H1, 1], F32, tag="b1")
            nc.scalar.dma_start(b1_s[:], b1[:])
            w2_s = cpool.tile([H1, HO], BF16, tag="w2")
            nc.scalar.dma_start(w2_s[:], w2[:])
            b2_s = cpool.tile([HO, 1], F32, tag="b2")
            nc.scalar.dma_start(b2_s[:], b2[:])
            w3_s = cpool.tile([HO, HO], BF16, tag="w3")
            nc.scalar.dma_start(w3_s[:], w3[:])
            b3w_s = cpool.tile([HO, TN], F32, tag="b3w")
            nc.scalar.dma_start(b3w_s[:], b3w[:])

            # All real matmuls are chained with same-engine ordering deps
            # so the PE executes them exactly in emission order — required
            # for ldweights=False (weight reuse from the previous matmul).
            prev_mm = [None]

            def mm(out_ap, lhsT_ap, rhs_ap, start, stop, ldw=True):
                m = nc.tensor.matmul(out_ap, lhsT_ap, rhs_ap,
                                     start=start, stop=stop)
                if not ldw:
                    m.ins.ldweights = False
                if prev_mm[0] is not None:
                    add_dep_helper(m.ins, prev_mm[0], sync=False,
                                   reason="pe program order")
                prev_mm[0] = m.ins
                return m

            # Warmup: dense dummy matmuls fill the NEFF startup ramp
            # (~12us of instruction loads + first DMAs) so the PE's HAM
            # throttle reaches full clock before the first real matmul.
            wsc = cpool.tile([128, TN], BF16, tag="wsc")
            nc.gpsimd.memset(wsc[:], 0.0)
            wp0 = ps1.tile([H1, TN], F32, tag="p1")
            wp1 = ps1.tile([H1, TN], F32, tag="p1")
            wp = [wp0, wp1]
            wfirst = nc.tensor.matmul(wp[0][:], wsc[:, 0:H1], wsc[:],
                                      start=True, stop=True)
            for i in range(1, 24):
                w_mm = nc.tensor.matmul(wp[i % 2][:], wsc[:, 0:H1], wsc[:],
                                        start=True, stop=True)
                w_mm.ins.ldweights = False
                add_dep_helper(w_mm.ins, wfirst.ins, sync=False,
                               reason="warmup weight reuse")

            # Pipeline over supertile PAIRS: at pair p emit L1(p),
            # L2(p-1), L3(p-2). Within a pair the two supertiles share
            # every LDWEIGHTS via ldweights=False (chunk-outer order).
            NP = NT // 2
            h1s: dict[int, object] = {}
            h2s: dict[int, object] = {}
            for p in range(NP + 2):
                if p < NP:
                    t0 = 2 * p
                    xmp = iopool.tile([128, 2, NKC, TN], BF16, tag="xm")
                    nc.sync.dma_start(xmp[:], xt_main[p])

                    p1a = ps1.tile([H1, TN], F32, tag="p1")
                    p1b = ps1.tile([H1, TN], F32, tag="p1")
                    for k in range(NKC):
                        mm(p1a[:], w1m_s[:, k, :], xmp[:, 0, k, :],
                           start=(k == 0), stop=False)
                        mm(p1b[:], w1m_s[:, k, :], xmp[:, 1, k, :],
                           start=(k == 0), stop=False, ldw=False)
                    mm(p1a[:], w1t_s[:], xtl_s[:, t0, :],
                       start=False, stop=True)
                    mm(p1b[:], w1t_s[:], xtl_s[:, t0 + 1, :],
                       start=False, stop=True, ldw=False)
                    for j, p1 in ((0, p1a), (1, p1b)):
                        h1 = apool.tile([H1, TN], BF16, tag="h1")
                        nc.scalar.activation(h1[:], p1[:], relu,
                                             bias=b1_s[:, 0:1])
                        h1s[t0 + j] = h1

                if 1 <= p < NP + 1:
                    s0 = 2 * (p - 1)
                    p2a = ps2.tile([HO, TN], F32, tag="p2")
                    p2b = ps2.tile([HO, TN], F32, tag="p2")
                    mm(p2a[:], w2_s[:], h1s.pop(s0)[:], start=True, stop=True)
                    mm(p2b[:], w2_s[:], h1s.pop(s0 + 1)[:],
                       start=True, stop=True, ldw=False)
                    for j, p2 in ((0, p2a), (1, p2b)):
                        h2 = apool.tile([HO, TN], BF16, tag="h2")
                        nc.scalar.activation(h2[:], p2[:], relu,
                                             bias=b2_s[:, 0:1])
                        h2s[s0 + j] = h2

                if p >= 2:
                    s0 = 2 * (p - 2)
                    p3a = ps3.tile([HO, TN], F32, tag="p3")
                    p3b = ps3.tile([HO, TN], F32, tag="p3")
                    mm(p3a[:], w3_s[:], h2s.pop(s0)[:], start=True, stop=True)
                    mm(p3b[:], w3_s[:], h2s.pop(s0 + 1)[:],
                       start=True, stop=True, ldw=False)
                    ot = apool.tile([HO, 2, TN], F32, tag="ot")
                    nc.vector.tensor_add(ot[:, 0, :], p3a[:], b3w_s[:])
                    nc.vector.tensor_add(ot[:, 1, :], p3b[:], b3w_s[:])
                    # scalar queue: keeps the in-order sync queue free
                    # for prefetch loads (a store waits on the L3 tail)
                    nc.scalar.dma_start(
                        yt[:, s0 * TN:(s0 + 2) * TN], ot[:]
                    )

    nc.compile()
    return nc


def _fold_conv_into_w1(conv_w: np.ndarray, w1: np.ndarray) -> np.ndarray:
    """W1f[784,100] such that x @ W1f == conv(x).reshape(B,676) @ w1."""
    c = np.zeros((NF, 26 * 26), dtype=np.float64)
    for di in range(3):
        for dj in range(3):
            ii, jj = np.meshgrid(np.arange(26), np.arange(26), indexing="ij")
            src = (ii + di) * 28 + (jj + dj)
            dst = ii * 26 + jj
            c[src.ravel(), dst.ravel()] += np.float64(conv_w[di, dj])
    return (c @ w1.astype(np.float64)).astype(np.float32)


def _prep_in_maps(x, conv_w, w1, b1, w2, b2, w3, b3):
    x = np.asarray(x, dtype=np.float32)
    conv_w = np.asarray(conv_w, dtype=np.float32)
    w1 = np.asarray(w1, dtype=np.float32)
    b1 = np.asarray(b1, dtype=np.float32)
    w2 = np.asarray(w2, dtype=np.float32)
    b2 = np.asarray(b2, dtype=np.float32)
    w3 = np.asarray(w3, dtype=np.float32)
    b3 = np.asarray(b3, dtype=np.float32)

    w1f = _fold_conv_into_w1(conv_w, w1)  # [784, 100]
    # main chunks: feature f = k*128 + p -> [128, NKC, H1]
    w1m = np.ascontiguousarray(
        w1f[: 128 * NKC].reshape(NKC, 128, H1).transpose(1, 0, 2)
    ).astype(NP_BF16)
    w1t = np.ascontiguousarray(w1f[128 * NKC:]).astype(NP_BF16)  # [16, 100]
    b1c = np.ascontiguousarray(b1.reshape(H1, 1))
    b2c = np.ascontiguousarray(b2.reshape(HO, 1))
    b3w = np.ascontiguousarray(np.broadcast_to(b3.reshape(HO, 1), (HO, TN)))

    shared = {
        "w1m": w1m, "w1t": w1t, "b1": b1c,
        "w2": np.ascontiguousarray(w2).astype(NP_BF16), "b2": b2c,
        "w3": np.ascontiguousarray(w3).astype(NP_BF16), "b3w": b3w,
    }

    xb = x.astype(NP_BF16)  # cast once, full batch
    NP2 = NT // 2
    in_maps = []
    for core in range(N_CORES):
        xc = xb[core * BC:(core + 1) * BC]  # [8192, 784] bf16
        # [NP2, feat, pair-half, TN] feature-major per supertile pair
        xct = xc.reshape(NP2, 2, TN, NF).transpose(0, 3, 1, 2)  # [NP2, NF, 2, TN]
        xt_main = np.ascontiguousarray(
            xct[:, : 128 * NKC].reshape(NP2, NKC, 128, 2, TN)
            .transpose(0, 2, 3, 1, 4)
        )  # [NP2, 128, 2, NKC, TN]
        # tails: [KT, NT, TN] with t = 2*pair + half
        xt_tail = np.ascontiguousarray(
            xct[:, 128 * NKC:].transpose(1, 0, 2, 3).reshape(KT, NT, TN)
        )
        in_maps.append({"xt_main": xt_main, "xt_tail": xt_tail, **shared})
    return in_maps


_NC = None


def _get_nc():
    global _NC
    if _NC is None:
        _NC = _build_nc()
    return _NC


def kernel(x, conv_w, w1, b1, w2, b2, w3, b3):
    in_maps = _prep_in_maps(x, conv_w, w1, b1, w2, b2, w3, b3)
    nc = _get_nc()
    res = run_bass_kernel_spmd(nc, in_maps, core_ids=list(range(N_CORES)))
    out = np.empty((B, HO), dtype=np.float32)
    for i in range(N_CORES):
        out[i * BC:(i + 1) * BC] = res.results[i]["yt"].T
    return out


if __name__ == "__main__":
    rng = np.random.default_rng(0)
    inputs = {
        "x": rng.standard_normal((B, NF), dtype=np.float32),
        "conv_w": np.ones((3, 3), dtype=np.float32),
        "w1": (rng.standard_normal((676, H1)) * 0.04).astype(np.float32),
        "b1": np.zeros(H1, dtype=np.float32),
        "w2": (rng.standard_normal((H1, HO)) * 0.1).astype(np.float32),
        "b2": np.zeros(HO, dtype=np.float32),
        "w3": (rng.standard_normal((HO, HO)) * 0.3).astype(np.float32),
        "b3": np.zeros(HO, dtype=np.float32),
    }
    out = kernel(**inputs)
    print(out.shape, out.dtype)


# revision 21
# speedup vs baseline: 1.1452x; 1.0201x over previous
"""Trainium2 Bass kernel for DigitConvolutionalModel.

Model: x[B,784] -> reshape 28x28 -> 3x3 valid conv (weights conv_w) ->
[B,676] -> Linear(676,100)+relu -> Linear(100,10)+relu -> Linear(10,10).

The conv is linear, so it folds into the first Linear: W1f = C @ w1 where
C[784,676] is the conv unfold matrix. The whole model becomes a 3-layer MLP
784 -> 100 -> 10 -> 10 with relu between layers.

Sharding: pure data parallel, batch split across 8 cores (8192 rows each).

Precision: matmuls in bf16 (PE streams fp32 at 1/4 rate, bf16 at full
rate), accumulation in fp32 PSUM, biases + output in fp32. x is cast to
bf16 host-side — bit-identical to casting on device, but halves the HBM
traffic, which is what the ridge regime wants (DMA ~36us/core vs PE
~31us/core).

On-chip layout: activations stay feature-major ([features, batch] on SBUF
partitions) end to end, so every matmul uses the weights in natural [in,out]
layout as the stationary operand and the batch streams as the moving free
dim:
    h1T[100,n] = sum_k W1f[k,:].T @ xT[k,n]      (K chunks of <=128)
    h2T[10,n]  = w2.T @ relu(h1T+b1)
    yT[10,n]   = w3.T @ relu(h2T+b2) + b3
The [10, B] output is stored feature-major (2KB-contiguous DMA runs; a
batch-major store would emit 40-byte descriptors) and transposed on host.

x is staged host-side into the feature-major tiled layout the DMA wants:
per 512-batch supertile [128, 6, 512] (features 0..767, 6KB contiguous per
partition); the [16, 512] feature tails (768..783) of all supertiles are
preloaded in one DMA.
"""

import numpy as np
import ml_dtypes

import concourse.bacc as bacc
import concourse.tile as tile
from concourse.tile import add_dep_helper
from concourse import mybir
from concourse.bass_utils import run_bass_kernel_spmd

N_CORES = 8
B = 65536
BC = B // N_CORES  # 8192 rows per core
TN = 512           # batch columns per supertile
NT = BC // TN      # 16 supertiles per core
NKC = 6            # full 128-feature chunks (0..767)
KT = 16            # tail features (768..783)
NF = 784
H1 = 100
HO = 10
F32 = mybir.dt.float32
BF16 = mybir.dt.bfloat16
NP_BF16 = ml_dtypes.bfloat16


def _build_nc():
    nc = bacc.Bacc(None, target_bir_lowering=False)

    xt_main = nc.dram_tensor(
        "xt_main", [NT // 2, 128, 2, NKC, TN], BF16, kind="ExternalInput"
    )
    xt_tail = nc.dram_tensor("xt_tail", [KT, NT, TN], BF16, kind="ExternalInput")
    w1m = nc.dram_tensor("w1m", [128, NKC, H1], BF16, kind="ExternalInput")
    w1t = nc.dram_tensor("w1t", [KT, H1], BF16, kind="ExternalInput")
    b1 = nc.dram_tensor("b1", [H1, 1], F32, kind="ExternalInput")
    w2 = nc.dram_tensor("w2", [H1, HO], BF16, kind="ExternalInput")
    b2 = nc.dram_tensor("b2", [HO, 1], F32, kind="ExternalInput")
    w3 = nc.dram_tensor("w3", [HO, HO], BF16, kind="ExternalInput")
    b3w = nc.dram_tensor("b3w", [HO, TN], F32, kind="ExternalInput")
    yt = nc.dram_tensor("yt", [HO, BC], F32, kind="ExternalOutput")

    relu = mybir.ActivationFunctionType.Relu

    with tile.TileContext(nc) as tc:
        with (
            tc.tile_pool(name="const", bufs=1) as cpool,
            tc.tile_pool(name="io", bufs=6) as iopool,
            tc.tile_pool(name="act", bufs=4) as apool,
            tc.tile_pool(name="ps1", bufs=4, space="PSUM") as ps1,
            tc.tile_pool(name="ps2", bufs=2, space="PSUM") as ps2,
            tc.tile_pool(name="ps3", bufs=2, space="PSUM") as ps3,
        ):
# Weights/consts go on the scalar HWDGE queue-set so they don't
            # delay the batch-data stream on the sync queue-set.
            w1m_s = cpool.tile([128, NKC, H1], BF16, tag="w1m")
            nc.scalar.dma_start(w1m_s[:], w1m[:])
            w1t_s = cpool.tile([KT, H1], BF16, tag="w1t")
            nc.scalar.dma_start(w1t_s[:], w1t[:])
            xtl_s = cpool.tile([KT, NT, TN], BF16, tag="xtl")
            nc.scalar.dma_start(xtl_s[:], xt_tail[:])
            b1_s = cpool.tile([H1, 1], F32, tag="b1")
            nc.scalar.dma_start(b1_s[:], b1[:])
            w2_s = cpool.tile([H1, HO], BF16, tag="w2")
            nc.scalar.dma_start(w2_s[:], w2[:])
            b2_s = cpool.tile([HO, 1], F32, tag="b2")
            nc.scalar.dma_start(b2_s[:], b2[:])
            w3_s = cpool.tile([HO, HO], BF16, tag="w3")
            nc.scalar.dma_start(w3_s[:], w3[:])
            b3w_s = cpool.tile([HO, TN], F32, tag="b3w")
            nc.scalar.dma_start(b3w_s[:], b3w[:])

            # All real matmuls are chained with same-engine ordering deps
            # so the PE executes them exactly in emission order — required
            # for ldweights=False (weight reuse from the previous matmul).
            prev_mm = [None]

            def mm(out_ap, lhsT_ap, rhs_ap, start, stop, ldw=True):
                m = nc.tensor.matmul(out_ap, lhsT_ap, rhs_ap,
                                     start=start, stop=stop)
                if not ldw:
                    m.ins.ldweights = False
                if prev_mm[0] is not None:
                    add_dep_helper(m.ins, prev_mm[0], sync=False,
                                   reason="pe program order")
                prev_mm[0] = m.ins
                return m

            # Warmup: dense dummy matmuls fill the NEFF startup ramp
            # (~12us of instruction loads + first DMAs) so the PE's HAM
            # throttle reaches full clock before the first real matmul.
            wsc = cpool.tile([128, TN], BF16, tag="wsc")
            nc.gpsimd.memset(wsc[:], 0.0)
            wp0 = ps1.tile([H1, TN], F32, tag="p1")
            wp1 = ps1.tile([H1, TN], F32, tag="p1")
            wp = [wp0, wp1]
            wfirst = nc.tensor.matmul(wp[0][:], wsc[:, 0:H1], wsc[:],
                                      start=True, stop=True)
            for i in range(1, 14):
                w_mm = nc.tensor.matmul(wp[i % 2][:], wsc[:, 0:H1], wsc[:],
                                        start=True, stop=True)
                w_mm.ins.ldweights = False
                add_dep_helper(w_mm.ins, wfirst.ins, sync=False,
                               reason="warmup weight reuse")

            # Pipeline over supertile PAIRS: at pair p emit L1(p),
            # L2(p-1), L3(p-2). Within a pair the two supertiles share
            # every LDWEIGHTS via ldweights=False (chunk-outer order).
            NP = NT // 2
            h1s: dict[int, object] = {}
            h2s: dict[int, object] = {}
            for p in range(NP + 2):
                if p < NP:
                    t0 = 2 * p
                    xmp = iopool.tile([128, 2, NKC, TN], BF16, tag="xm")
                    nc.sync.dma_start(xmp[:], xt_main[p])

                    p1a = ps1.tile([H1, TN], F32, tag="p1")
                    p1b = ps1.tile([H1, TN], F32, tag="p1")
                    for k in range(NKC):
                        mm(p1a[:], w1m_s[:, k, :], xmp[:, 0, k, :],
                           start=(k == 0), stop=False)
                        mm(p1b[:], w1m_s[:, k, :], xmp[:, 1, k, :],
                           start=(k == 0), stop=False, ldw=False)
                    mm(p1a[:], w1t_s[:], xtl_s[:, t0, :],
                       start=False, stop=True)
                    mm(p1b[:], w1t_s[:], xtl_s[:, t0 + 1, :],
                       start=False, stop=True, ldw=False)
                    for j, p1 in ((0, p1a), (1, p1b)):
                        h1 = apool.tile([H1, TN], BF16, tag="h1")
                        nc.scalar.activation(h1[:], p1[:], relu,
                                             bias=b1_s[:, 0:1])
                        h1s[t0 + j] = h1

                if 1 <= p < NP + 1:
                    s0 = 2 * (p - 1)
                    p2a = ps2.tile([HO, TN], F32, tag="p2")
                    p2b = ps2.tile([HO, TN], F32, tag="p2")
                    mm(p2a[:], w2_s[:], h1s.pop(s0)[:], start=True, stop=True)
                    mm(p2b[:], w2_s[:], h1s.pop(s0 + 1)[:],
                       start=True, stop=True, ldw=False)
                    for j, p2 in ((0, p2a), (1, p2b)):
                        h2 = apool.tile([HO, TN], BF16, tag="h2")
                        nc.scalar.activation(h2[:], p2[:], relu,
                                             bias=b2_s[:, 0:1])
                        h2s[s0 + j] = h2

                if p >= 2:
                    s0 = 2 * (p - 2)
                    p3a = ps3.tile([HO, TN], F32, tag="p3")
                    p3b = ps3.tile([HO, TN], F32, tag="p3")
                    mm(p3a[:], w3_s[:], h2s.pop(s0)[:], start=True, stop=True)
                    mm(p3b[:], w3_s[:], h2s.pop(s0 + 1)[:],
                       start=True, stop=True, ldw=False)
                    ot = apool.tile([HO, 2, TN], F32, tag="ot")
                    nc.vector.tensor_add(ot[:, 0, :], p3a[:], b3w_s[:])
                    nc.vector.tensor_add(ot[:, 1, :], p3b[:], b3w_s[:])
                    # gpsimd (SWDGE): stores wait on the L3 tail, so they
                    # must not sit in the sync (loads) or scalar (ACT
                    # relus) in-order streams
                    nc.gpsimd.dma_start(
                        yt[:, s0 * TN:(s0 + 2) * TN], ot[:]
                    )

    nc.compile()
    return nc


def _fold_conv_into_w1(conv_w: np.ndarray, w1: np.ndarray) -> np.ndarray:
    """W1f[784,100] such that x @ W1f == conv(x).reshape(B,676) @ w1."""
    c = np.zeros((NF, 26 * 26), dtype=np.float64)
    for di in range(3):
        for dj in range(3):
            ii, jj = np.meshgrid(np.arange(26), np.arange(26), indexing="ij")
            src = (ii + di) * 28 + (jj + dj)
            dst = ii * 26 + jj
            c[src.ravel(), dst.ravel()] += np.float64(conv_w[di, dj])
    return (c @ w1.astype(np.float64)).astype(np.float32)


def _prep_in_maps(x, conv_w, w1, b1, w2, b2, w3, b3):
    x = np.asarray(x, dtype=np.float32)
    conv_w = np.asarray(conv_w, dtype=np.float32)
    w1 = np.asarray(w1, dtype=np.float32)
    b1 = np.asarray(b1, dtype=np.float32)
    w2 = np.asarray(w2, dtype=np.float32)
    b2 = np.asarray(b2, dtype=np.float32)
    w3 = np.asarray(w3, dtype=np.float32)
    b3 = np.asarray(b3, dtype=np.float32)

    w1f = _fold_conv_into_w1(conv_w, w1)  # [784, 100]
    # main chunks: feature f = k*128 + p -> [128, NKC, H1]
    w1m = np.ascontiguousarray(
        w1f[: 128 * NKC].reshape(NKC, 128, H1).transpose(1, 0, 2)
    ).astype(NP_BF16)
    w1t = np.ascontiguousarray(w1f[128 * NKC:]).astype(NP_BF16)  # [16, 100]
    b1c = np.ascontiguousarray(b1.reshape(H1, 1))
    b2c = np.ascontiguousarray(b2.reshape(HO, 1))
    b3w = np.ascontiguousarray(np.broadcast_to(b3.reshape(HO, 1), (HO, TN)))

    shared = {
        "w1m": w1m, "w1t": w1t, "b1": b1c,
        "w2": np.ascontiguousarray(w2).astype(NP_BF16), "b2": b2c,
        "w3": np.ascontiguousarray(w3).astype(NP_BF16), "b3w": b3w,
    }

    xb = x.astype(NP_BF16)  # cast once, full batch
    NP2 = NT // 2
    in_maps = []
    for core in range(N_CORES):
        xc = xb[core * BC:(core + 1) * BC]  # [8192, 784] bf16
        # [NP2, feat, pair-half, TN] feature-major per supertile pair
        xct = xc.reshape(NP2, 2, TN, NF).transpose(0, 3, 1, 2)  # [NP2, NF, 2, TN]
        xt_main = np.ascontiguousarray(
            xct[:, : 128 * NKC].reshape(NP2, NKC, 128, 2, TN)
            .transpose(0, 2, 3, 1, 4)
        )  # [NP2, 128, 2, NKC, TN]
        # tails: [KT, NT, TN] with t = 2*pair + half
        xt_tail = np.ascontiguousarray(
            xct[:, 128 * NKC:].transpose(1, 0, 2, 3).reshape(KT, NT, TN)
        )
        in_maps.append({"xt_main": xt_main, "xt_tail": xt_tail, **shared})
    return in_maps


_NC = None


def _get_nc():
    global _NC
    if _NC is None:
        _NC = _build_nc()
    return _NC


def kernel(x, conv_w, w1, b1, w2, b2, w3, b3):
    in_maps = _prep_in_maps(x, conv_w, w1, b1, w2, b2, w3, b3)
    nc = _get_nc()
    res = run_bass_kernel_spmd(nc, in_maps, core_ids=list(range(N_CORES)))
    out = np.empty((B, HO), dtype=np.float32)
    for i in range(N_CORES):
        out[i * BC:(i + 1) * BC] = res.results[i]["yt"].T
    return out


if __name__ == "__main__":
    rng = np.random.default_rng(0)
    inputs = {
        "x": rng.standard_normal((B, NF), dtype=np.float32),
        "conv_w": np.ones((3, 3), dtype=np.float32),
        "w1": (rng.standard_normal((676, H1)) * 0.04).astype(np.float32),
        "b1": np.zeros(H1, dtype=np.float32),
        "w2": (rng.standard_normal((H1, HO)) * 0.1).astype(np.float32),
        "b2": np.zeros(HO, dtype=np.float32),
        "w3": (rng.standard_normal((HO, HO)) * 0.3).astype(np.float32),
        "b3": np.zeros(HO, dtype=np.float32),
    }
    out = kernel(**inputs)
    print(out.shape, out.dtype)


# revision 26
# speedup vs baseline: 1.1646x; 1.0169x over previous
"""Trainium2 Bass kernel for DigitConvolutionalModel.

Model: x[B,784] -> reshape 28x28 -> 3x3 valid conv (weights conv_w) ->
[B,676] -> Linear(676,100)+relu -> Linear(100,10)+relu -> Linear(10,10).

The conv is linear, so it folds into the first Linear: W1f = C @ w1 where
C[784,676] is the conv unfold matrix. The whole model becomes a 3-layer MLP
784 -> 100 -> 10 -> 10 with relu between layers.

Sharding: pure data parallel, batch split across 8 cores (8192 rows each).

Precision: matmuls in bf16 (PE streams fp32 at 1/4 rate, bf16 at full
rate), accumulation in fp32 PSUM, biases + output in fp32. x is cast to
bf16 host-side — bit-identical to casting on device, but halves the HBM
traffic, which is what the ridge regime wants (DMA ~36us/core vs PE
~31us/core).

On-chip layout: activations stay feature-major ([features, batch] on SBUF
partitions) end to end, so every matmul uses the weights in natural [in,out]
layout as the stationary operand and the batch streams as the moving free
dim:
    h1T[100,n] = sum_k W1f[k,:].T @ xT[k,n]      (K chunks of <=128)
    h2T[10,n]  = w2.T @ relu(h1T+b1)
    yT[10,n]   = w3.T @ relu(h2T+b2) + b3
The [10, B] output is stored feature-major (2KB-contiguous DMA runs; a
batch-major store would emit 40-byte descriptors) and transposed on host.

x is staged host-side into the feature-major tiled layout the DMA wants:
per 512-batch supertile [128, 6, 512] (features 0..767, 6KB contiguous per
partition); the [16, 512] feature tails (768..783) of all supertiles are
preloaded in one DMA.
"""

import numpy as np
import ml_dtypes

import concourse.bacc as bacc
import concourse.tile as tile
from concourse.tile import add_dep_helper
from concourse import mybir
from concourse.bass_utils import run_bass_kernel_spmd

N_CORES = 8
B = 65536
BC = B // N_CORES  # 8192 rows per core
TN = 512           # batch columns per supertile
NT = BC // TN      # 16 supertiles per core
NKC = 6            # full 128-feature chunks (0..767)
KT = 16            # tail features (768..783)
NF = 784
H1 = 100
HO = 10
F32 = mybir.dt.float32
BF16 = mybir.dt.bfloat16
NP_BF16 = ml_dtypes.bfloat16

# packed weight blob column layout (bf16 columns)
_C_W1M = 0                      # [128, 600]  w1m chunks
_C_W1T = 600                    # [16, 100]   w1t
_C_W2 = 700                     # [100, 10]   w2
_C_W3 = 710                     # [10, 10]    w3
_C_B1 = 720                     # [100, 2]    b1 as f32 byte-pairs
_C_B2 = 722                     # [10, 2]     b2
_C_B3W = 724                    # [10, 1024]  b3 broadcast [10, TN] f32
WBW = _C_B3W + 2 * TN


def _build_nc():
    nc = bacc.Bacc(None, target_bir_lowering=False)

    xt_main = nc.dram_tensor(
        "xt_main", [NT // 2, 128, 2, NKC, TN], BF16, kind="ExternalInput"
    )
    xt_tail = nc.dram_tensor("xt_tail", [KT, NT, TN], BF16, kind="ExternalInput")
    # all weights + biases packed into one [128, WBW] bf16 blob (one DMA);
    # f32 fields are stored as bf16 byte-pairs and bitcast on the SBUF side
    wblob = nc.dram_tensor("wblob", [128, WBW], BF16, kind="ExternalInput")
    yt = nc.dram_tensor("yt", [HO, BC], F32, kind="ExternalOutput")

    relu = mybir.ActivationFunctionType.Relu

    with tile.TileContext(nc) as tc:
        with (
            tc.tile_pool(name="const", bufs=1) as cpool,
            tc.tile_pool(name="io", bufs=6) as iopool,
            tc.tile_pool(name="act", bufs=4) as apool,
            tc.tile_pool(name="ps1", bufs=4, space="PSUM") as ps1,
            tc.tile_pool(name="ps2", bufs=2, space="PSUM") as ps2,
            tc.tile_pool(name="ps3", bufs=2, space="PSUM") as ps3,
        ):
# One weights DMA + the tails DMA, first in the sync queue so
            # they land before the first xm pair.
            wb_s = cpool.tile([128, WBW], BF16, tag="wb")
            nc.sync.dma_start(wb_s[:], wblob[:])
            xtl_s = cpool.tile([KT, NT, TN], BF16, tag="xtl")
            nc.sync.dma_start(xtl_s[:], xt_tail[:])

            w1t_ap = wb_s[0:KT, _C_W1T:_C_W1T + H1]
            w2_ap = wb_s[0:H1, _C_W2:_C_W2 + HO]
            w3_ap = wb_s[0:HO, _C_W3:_C_W3 + HO]
            b1_ap = wb_s[0:H1, _C_B1:_C_B1 + 2].bitcast(F32)
            b2_ap = wb_s[0:HO, _C_B2:_C_B2 + 2].bitcast(F32)
            b3w_ap = wb_s[0:HO, _C_B3W:_C_B3W + 2 * TN].bitcast(F32)

            # All real matmuls are chained with same-engine ordering deps
            # so the PE executes them exactly in emission order — required
            # for ldweights=False (weight reuse from the previous matmul).
            prev_mm = [None]

            def mm(out_ap, lhsT_ap, rhs_ap, start, stop, ldw=True):
                m = nc.tensor.matmul(out_ap, lhsT_ap, rhs_ap,
                                     start=start, stop=stop)
                if not ldw:
                    m.ins.ldweights = False
                if prev_mm[0] is not None:
                    add_dep_helper(m.ins, prev_mm[0], sync=False,
                                   reason="pe program order")
                prev_mm[0] = m.ins
                return m

            # Warmup: dense dummy matmuls fill the NEFF startup ramp
            # (~12us of instruction loads + first DMAs) so the PE's HAM
            # throttle reaches full clock before the first real matmul.
            wsc = cpool.tile([128, TN], BF16, tag="wsc")
            nc.gpsimd.memset(wsc[:], 0.0)
            wp0 = ps1.tile([H1, TN], F32, tag="p1")
            wp1 = ps1.tile([H1, TN], F32, tag="p1")
            wp = [wp0, wp1]
            wfirst = nc.tensor.matmul(wp[0][:], wsc[:, 0:H1], wsc[:],
                                      start=True, stop=True)
            for i in range(1, 14):
                w_mm = nc.tensor.matmul(wp[i % 2][:], wsc[:, 0:H1], wsc[:],
                                        start=True, stop=True)
                w_mm.ins.ldweights = False
                add_dep_helper(w_mm.ins, wfirst.ins, sync=False,
                               reason="warmup weight reuse")

            # Pipeline over supertile PAIRS: at pair p emit L1(p),
            # L2(p-1), L3(p-2). Within a pair the two supertiles share
            # every LDWEIGHTS via ldweights=False (chunk-outer order).
            NP = NT // 2
            h1s: dict[int, object] = {}
            h2s: dict[int, object] = {}
            for p in range(NP + 2):
                if p < NP:
                    t0 = 2 * p
                    xmp = iopool.tile([128, 2, NKC, TN], BF16, tag="xm")
                    nc.sync.dma_start(xmp[:], xt_main[p])

                    p1a = ps1.tile([H1, TN], F32, tag="p1")
                    p1b = ps1.tile([H1, TN], F32, tag="p1")
                    for k in range(NKC):
                        mm(p1a[:], wb_s[:, k * H1:(k + 1) * H1], xmp[:, 0, k, :],
                           start=(k == 0), stop=False)
                        mm(p1b[:], wb_s[:, k * H1:(k + 1) * H1], xmp[:, 1, k, :],
                           start=(k == 0), stop=False, ldw=False)
                    mm(p1a[:], w1t_ap, xtl_s[:, t0, :],
                       start=False, stop=True)
                    mm(p1b[:], w1t_ap, xtl_s[:, t0 + 1, :],
                       start=False, stop=True, ldw=False)
                    for j, p1 in ((0, p1a), (1, p1b)):
                        h1 = apool.tile([H1, TN], BF16, tag="h1")
                        nc.scalar.activation(h1[:], p1[:], relu,
                                             bias=b1_ap)
                        h1s[t0 + j] = h1

                if 1 <= p < NP + 1:
                    s0 = 2 * (p - 1)
                    p2a = ps2.tile([HO, TN], F32, tag="p2")
                    p2b = ps2.tile([HO, TN], F32, tag="p2")
                    mm(p2a[:], w2_ap, h1s.pop(s0)[:], start=True, stop=True)
                    mm(p2b[:], w2_ap, h1s.pop(s0 + 1)[:],
                       start=True, stop=True, ldw=False)
                    for j, p2 in ((0, p2a), (1, p2b)):
                        h2 = apool.tile([HO, TN], BF16, tag="h2")
                        nc.scalar.activation(h2[:], p2[:], relu,
                                             bias=b2_ap)
                        h2s[s0 + j] = h2

                if p >= 2:
                    s0 = 2 * (p - 2)
                    p3a = ps3.tile([HO, TN], F32, tag="p3")
                    p3b = ps3.tile([HO, TN], F32, tag="p3")
                    mm(p3a[:], w3_ap, h2s.pop(s0)[:], start=True, stop=True)
                    mm(p3b[:], w3_ap, h2s.pop(s0 + 1)[:],
                       start=True, stop=True, ldw=False)
                    ot = apool.tile([HO, 2, TN], F32, tag="ot")
                    nc.vector.tensor_add(ot[:, 0, :], p3a[:], b3w_ap)
                    nc.vector.tensor_add(ot[:, 1, :], p3b[:], b3w_ap)
                    # gpsimd (SWDGE): stores wait on the L3 tail, so they
                    # must not sit in the sync (loads) or scalar (ACT
                    # relus) in-order streams
                    nc.gpsimd.dma_start(
                        yt[:, s0 * TN:(s0 + 2) * TN], ot[:]
                    )

    nc.compile()
    return nc


def _fold_conv_into_w1(conv_w: np.ndarray, w1: np.ndarray) -> np.ndarray:
    """W1f[784,100] such that x @ W1f == conv(x).reshape(B,676) @ w1."""
    c = np.zeros((NF, 26 * 26), dtype=np.float64)
    for di in range(3):
        for dj in range(3):
            ii, jj = np.meshgrid(np.arange(26), np.arange(26), indexing="ij")
            src = (ii + di) * 28 + (jj + dj)
            dst = ii * 26 + jj
            c[src.ravel(), dst.ravel()] += np.float64(conv_w[di, dj])
    return (c @ w1.astype(np.float64)).astype(np.float32)


def _prep_in_maps(x, conv_w, w1, b1, w2, b2, w3, b3):
    x = np.asarray(x, dtype=np.float32)
    conv_w = np.asarray(conv_w, dtype=np.float32)
    w1 = np.asarray(w1, dtype=np.float32)
    b1 = np.asarray(b1, dtype=np.float32)
    w2 = np.asarray(w2, dtype=np.float32)
    b2 = np.asarray(b2, dtype=np.float32)
    w3 = np.asarray(w3, dtype=np.float32)
    b3 = np.asarray(b3, dtype=np.float32)

    w1f = _fold_conv_into_w1(conv_w, w1)  # [784, 100]
    # main chunks: feature f = k*128 + p -> [128, NKC*H1]
    w1m = np.ascontiguousarray(
        w1f[: 128 * NKC].reshape(NKC, 128, H1).transpose(1, 0, 2)
    ).astype(NP_BF16).reshape(128, NKC * H1)
    w1t = w1f[128 * NKC:].astype(NP_BF16)  # [16, 100]

    blob = np.zeros((128, WBW), np.uint16)
    blob[:, _C_W1M:_C_W1M + NKC * H1] = w1m.view(np.uint16)
    blob[0:KT, _C_W1T:_C_W1T + H1] = w1t.view(np.uint16)
    blob[0:H1, _C_W2:_C_W2 + HO] = w2.astype(NP_BF16).view(np.uint16)
    blob[0:HO, _C_W3:_C_W3 + HO] = w3.astype(NP_BF16).view(np.uint16)
    blob[0:H1, _C_B1:_C_B1 + 2] = b1.reshape(H1, 1).view(np.uint16)
    blob[0:HO, _C_B2:_C_B2 + 2] = b2.reshape(HO, 1).view(np.uint16)
    b3w = np.ascontiguousarray(
        np.broadcast_to(b3.reshape(HO, 1), (HO, TN))
    ).view(np.uint16)
    blob[0:HO, _C_B3W:_C_B3W + 2 * TN] = b3w
    shared = {"wblob": blob.view(NP_BF16)}

    xb = x.astype(NP_BF16)  # cast once, full batch
    NP2 = NT // 2
    in_maps = []
    for core in range(N_CORES):
        xc = xb[core * BC:(core + 1) * BC]  # [8192, 784] bf16
        # [NP2, feat, pair-half, TN] feature-major per supertile pair
        xct = xc.reshape(NP2, 2, TN, NF).transpose(0, 3, 1, 2)  # [NP2, NF, 2, TN]
        xt_main = np.ascontiguousarray(
            xct[:, : 128 * NKC].reshape(NP2, NKC, 128, 2, TN)
            .transpose(0, 2, 3, 1, 4)
        )  # [NP2, 128, 2, NKC, TN]
        # tails: [KT, NT, TN] with t = 2*pair + half
        xt_tail = np.ascontiguousarray(
            xct[:, 128 * NKC:].transpose(1, 0, 2, 3).reshape(KT, NT, TN)
        )
        in_maps.append({"xt_main": xt_main, "xt_tail": xt_tail, **shared})
    return in_maps


_NC = None


def _get_nc():
    global _NC
    if _NC is None:
        _NC = _build_nc()
    return _NC


def kernel(x, conv_w, w1, b1, w2, b2, w3, b3):
    in_maps = _prep_in_maps(x, conv_w, w1, b1, w2, b2, w3, b3)
    nc = _get_nc()
    res = run_bass_kernel_spmd(nc, in_maps, core_ids=list(range(N_CORES)))
    out = np.empty((B, HO), dtype=np.float32)
    for i in range(N_CORES):
        out[i * BC:(i + 1) * BC] = res.results[i]["yt"].T
    return out


if __name__ == "__main__":
    rng = np.random.default_rng(0)
    inputs = {
        "x": rng.standard_normal((B, NF), dtype=np.float32),
        "conv_w": np.ones((3, 3), dtype=np.float32),
        "w1": (rng.standard_normal((676, H1)) * 0.04).astype(np.float32),
        "b1": np.zeros(H1, dtype=np.float32),
        "w2": (rng.standard_normal((H1, HO)) * 0.1).astype(np.float32),
        "b2": np.zeros(HO, dtype=np.float32),
        "w3": (rng.standard_normal((HO, HO)) * 0.3).astype(np.float32),
        "b3": np.zeros(HO, dtype=np.float32),
    }
    out = kernel(**inputs)
    print(out.shape, out.dtype)
